# revision 19
# baseline (speedup 1.0000x reference)
"""Dual multi-head attention (two attention paths, elementwise-fused) for
Trainium2, SPMD over 8 NeuronCores.

Sharding: core c -> batch b = c//4, head-quad hq = c%4 (4 of 16 heads).
Each core computes both attention paths for its (b, head-quad) shard:
  - Q/K projections in transposed layout [feat, seq] (bf16) with RoPE fused
    into the PSUM evacuation (swap-copy trick).
  - scores^T[k, q] per head via row-group-packed K=64 bf16 matmuls
    (two heads concurrently in the 128x128 PE array).
  - causal/any masking via additive -1e9 bias tiles on PSUM blocks that the
    host classifies as "mixed"; blocks that are fully masked are skipped
    entirely (device outputs are pre-zeroed by the runtime).
  - exp on ScalarE (scale=1/sqrt(d) folded in) -> unnormalized weights
    (float32r) which are both DMA'd out (packed) and fed to the ctx matmul.
  - ctx^T = [V | 1]^T @ expS accumulated over k-blocks (f32r, M=65; row 64
    gives the softmax denominator).
  - the two paths' unnormalized ctx are multiplied elementwise, scaled by
    1/(d1*d2) (broadcast via a tiny K=2 selector matmul), then projected
    through the WO column-shard. Host sums the 4 head-quad partials per batch.
  - softmax normalization of the exported weights happens on the host
    (w = expS^T / den), as does the [k,q] -> [q,k] transpose.
"""

import math
import os

import ml_dtypes
import numpy as np

import concourse.bass as bass
import concourse.mybir as mybir
import concourse.tile as tile
from concourse.bass_utils import run_bass_kernel_spmd

F32 = mybir.dt.float32
F32R = mybir.dt.float32r
BF16 = mybir.dt.bfloat16

N_HEAD = 16
N_EMBD = 1024
HEAD_DIM = 64
N_CORES = 8
HQ = 4  # heads per core
FSH = HQ * HEAD_DIM  # 256 features per core shard
QC = 512  # q chunk (columns per matmul)
KB = 128  # k block

PROFILE = False
LAST_EXEC_NS = None

_prog_cache = {}


# --------------------------------------------------------------------------
# wait legalization: CoreV3 ISA has a single sync-wait slot per instruction
# --------------------------------------------------------------------------
_waitfix_counter = [0]


def _legalize_waits(nc, limit=1):
    n_inserted = 0
    for bb in nc.main_func.blocks:
        insts = bb.instructions
        i = 0
        while i < len(insts):
            inst = insts[i]
            si = inst.sync_info
            if si is None or not si.on_wait:
                i += 1
                continue
            waits = list(si.on_wait)
            if len(waits) <= limit:
                i += 1
                continue
            excess, keep = waits[:-limit], waits[-limit:]
            nops = []
            for j in range(0, len(excess), limit):
                chunk = excess[j : j + limit]
                _waitfix_counter[0] += 1
                nop = mybir.InstNoOp(
                    name=f"I-waitfix-{_waitfix_counter[0]}",
                    engine=inst.engine,
                    sync_info=mybir.SyncInfo(on_wait=chunk, on_update=[]),
                )
                nc.register_instruction(nop)
                nops.append(nop)
            inst.sync_info = mybir.SyncInfo(on_wait=keep, on_update=list(si.on_update))
            for k, nop in enumerate(nops):
                insts.insert(i + k, nop)
            n_inserted += len(nops)
            i += len(nops) + 1
    return n_inserted


# --------------------------------------------------------------------------
# mask block classification and piece planning (host side)
# --------------------------------------------------------------------------
def _classify_mask(mask2d):
    """mask2d: [S, S] bool, mask2d[q, k]. Blocks at 128x128 granularity in
    (kb, qb) orientation. Returns kstat[kb][qb] in {0=skip,1=full,2+m=mixed}
    and the list of unique mixed bias tiles [128k, 128q] (fp32, (mT-1)*1e9)."""
    S = mask2d.shape[0]
    nb = S // KB
    kstat = [[0] * nb for _ in range(nb)]
    tiles = []
    tile_idx = {}
    for kb in range(nb):
        for qb in range(nb):
            blk = mask2d[qb * KB : (qb + 1) * KB, kb * KB : (kb + 1) * KB]
            if blk.all():
                kstat[kb][qb] = 1
            elif not blk.any():
                kstat[kb][qb] = 0
            else:
                bt = ((blk.T.astype(np.float32)) - 1.0) * 1e9
                key = bt.tobytes()
                if key not in tile_idx:
                    tile_idx[key] = len(tiles)
                    tiles.append(bt)
                kstat[kb][qb] = 2 + tile_idx[key]
    return kstat, tiles


def _plan_pieces(kstat, S):
    """Plan per-(qc) lists of pieces. A piece is one (kb, qc) score/exp/ctx
    unit covering a contiguous run of q-blocks. Ensures the first piece of
    every qc covers the union span of all later pieces (PSUM has_written
    safety). Returns (pieces_by_qc, npack, piece_offsets).

    piece = dict(kb, qb0, qb1, mixed=[(qb, tile_idx)], poff)
    """
    nqc = S // QC
    nb = S // KB
    pieces_by_qc = []
    poff = 0
    for qc in range(nqc):
        qb_lo, qb_hi = qc * 4, qc * 4 + 4
        plist = []
        for kb in range(nb):
            qbs = [qb for qb in range(qb_lo, qb_hi) if kstat[kb][qb] != 0]
            if not qbs:
                continue
            plist.append({"kb": kb, "qb0": min(qbs), "qb1": max(qbs)})
        if not plist:
            pieces_by_qc.append([])
            continue
        # union span; first piece must cover it
        u0 = min(p["qb0"] for p in plist)
        u1 = max(p["qb1"] for p in plist)
        plist[0]["qb0"], plist[0]["qb1"] = u0, u1
        for p in plist:
            mixed = []
            for qb in range(p["qb0"], p["qb1"] + 1):
                st = kstat[p["kb"]][qb]
                if st >= 2:
                    mixed.append((qb, st - 2))
                elif st == 0:
                    mixed.append((qb, -1))  # inside-run skip: zero it out
            p["mixed"] = mixed
            p["poff"] = poff
            poff += KB * (p["qb1"] - p["qb0"] + 1) * KB
        pieces_by_qc.append(plist)
    return pieces_by_qc, poff


def _need_zero_tile(pieces_by_qc):
    return any(
        ti == -1 for pl in pieces_by_qc for p in pl for (_, ti) in p["mixed"]
    )


# --------------------------------------------------------------------------
# device program
# --------------------------------------------------------------------------
def _build_program(S, pieces_by_qc, npack, n_mtiles):
    nqc = S // QC
    nkb = S // KB
    nc = bass.Bass()

    dp = nc.declare_dram_parameter
    xq = dp("xq", [128, 8 * S], BF16, isOutput=False)
    xk = dp("xk", [128, 8 * S], BF16, isOutput=False)
    xv = dp("xv", [128, 8 * S], F32R, isOutput=False)
    wq = [dp(f"wq{p}", [128, 8 * FSH], BF16, isOutput=False) for p in (1, 2)]
    wk = [dp(f"wk{p}", [128, 8 * FSH], BF16, isOutput=False) for p in (1, 2)]
    wv = [dp(f"wv{p}", [128, 8 * FSH], F32R, isOutput=False) for p in (1, 2)]
    wo = dp("wo", [128, 2 * N_EMBD], F32R, isOutput=False)
    ropec = dp("ropec", [128, S], BF16, isOutput=False)
    ropes = dp("ropes", [128, S], BF16, isOutput=False)
    sel = dp("sel", [1, 256], F32R, isOutput=False)
    onesv = dp("onesv", [128, nkb * HQ], F32R, isOutput=False)
    if n_mtiles:
        mtin = dp("mtiles", [128, n_mtiles * 128], F32, isOutput=False)
    wout = [dp(f"w{p}p", [HQ, npack], F32R, isOutput=True) for p in (1, 2)]
    outp = dp("outp", [S, N_EMBD], F32, isOutput=True)
    dent = dp("den", [8, S], F32, isOutput=True)

    with tile.TileContext(nc) as tc:
        with (
            tc.tile_pool(name="static", bufs=1) as st,
            tc.tile_pool(name="work", bufs=3) as wk_pool,
            tc.tile_pool(name="psum", bufs=2, space="PSUM") as pp,
        ):
            # ---- static tiles (live across all phases) ----
            vt = [st.tile([128, nkb, HQ, 65], F32R, name=f"vt{p}") for p in (0, 1)]
            ctx = [st.tile([128, S], F32R, name=f"ctx{t}") for t in (0, 1)]
            # den rows at 32-aligned partitions: den_ab[path][32*h] = denom
            den_ab = [st.tile([128, S], F32, name=f"den{p}") for p in (0, 1)]
            sel_sb = st.tile([1, 256], F32R, name="sel_sb")
            if n_mtiles:
                mt_sb = st.tile([128, n_mtiles, 128], F32, name="mt_sb")

            nc.sync.dma_start(out=sel_sb, in_=sel[:])
            if n_mtiles:
                nc.sync.dma_start(
                    out=mt_sb,
                    in_=mtin[:].rearrange("p (m c) -> p m c", m=n_mtiles),
                )
            # ones columns of V tiles (col 64 of each [128, 65] group)
            for p in (0, 1):
                nc.sync.dma_start(
                    out=vt[p][:, :, :, 64:65],
                    in_=onesv[:].rearrange("p (k h) -> p k h", k=nkb)[:, :, :, None],
                )

            psum_ctr = [0]

            def psum_tile(tag=None):
                # pS: [128,1024] double-bank tiles (score pairs); pCA/pCB:
                # [128,512] (ctx accumulators / general round-robin)
                if tag is None:
                    tag = ("pCA", "pCB")[psum_ctr[0] % 2]
                    psum_ctr[0] += 1
                w = 2 * QC if tag == "pS" else QC
                return pp.tile([128, w], F32, tag=tag, name=tag)

            # ---- phase A: projections ----
            def proj_qk(x_sb, w_sb, out_tile, rc_sb, rs_sb):
                for fb in range(2):
                    for qc in range(nqc):
                        ps = psum_tile()
                        for e in range(8):
                            nc.tensor.matmul(
                                ps[:],
                                w_sb[:, e, fb * 128 : (fb + 1) * 128],
                                x_sb[:, e, qc * QC : (qc + 1) * QC],
                                start=(e == 0),
                                stop=(e == 7),
                            )
                        tmp = wk_pool.tile([128, QC], BF16, tag="tmp")
                        nc.scalar.copy(tmp[:], ps[:])
                        tsw = wk_pool.tile([128, QC], BF16, tag="tsw")
                        for blk in range(4):
                            src = blk ^ 1
                            nc.gpsimd.tensor_copy(
                                tsw[blk * 32 : (blk + 1) * 32, :],
                                tmp[src * 32 : (src + 1) * 32, :],
                            )
                        qsl = slice(qc * QC, (qc + 1) * QC)
                        t1 = wk_pool.tile([128, QC], BF16, tag="t1")
                        nc.vector.tensor_mul(t1[:], tmp[:], rc_sb[:, qsl])
                        t2 = wk_pool.tile([128, QC], BF16, tag="t2")
                        nc.vector.tensor_mul(t2[:], tsw[:], rs_sb[:, qsl])
                        nc.vector.tensor_add(out_tile[:, fb, qsl], t1[:], t2[:])

            with tc.tile_pool(name="pqk", bufs=1) as pqk:
                qt = [pqk.tile([128, 2, S], BF16, name=f"qt{p}") for p in (0, 1)]
                kt = [pqk.tile([128, 2, S], BF16, name=f"kt{p}") for p in (0, 1)]

                for which, x_in, w_in, dst in (
                    ("q", xq, wq, qt),
                    ("k", xk, wk, kt),
                ):
                    with tc.tile_pool(name=f"px_{which}", bufs=1) as px:
                        x_sb = px.tile([128, 8, S], BF16, name=f"x{which}_sb")
                        nc.sync.dma_start(
                            out=x_sb, in_=x_in[:].rearrange("p (e q) -> p e q", e=8)
                        )
                        rc_sb = px.tile([128, S], BF16, name=f"rc_{which}")
                        rs_sb = px.tile([128, S], BF16, name=f"rs_{which}")
                        nc.sync.dma_start(out=rc_sb, in_=ropec[:])
                        nc.sync.dma_start(out=rs_sb, in_=ropes[:])
                        for p in (0, 1):
                            w_sb = px.tile(
                                [128, 8, FSH], BF16, name=f"w{which}_sb", tag="wqk"
                            )
                            nc.sync.dma_start(
                                out=w_sb,
                                in_=w_in[p][:].rearrange("p (e f) -> p e f", e=8),
                            )
                            proj_qk(x_sb, w_sb, dst[p], rc_sb, rs_sb)

                with tc.tile_pool(name="px_v", bufs=1) as px:
                    xv_sb = px.tile([128, 8, S], F32R, name="xv_sb")
                    nc.sync.dma_start(
                        out=xv_sb, in_=xv[:].rearrange("p (e q) -> p e q", e=8)
                    )
                    for p in (0, 1):
                        wv_sb = px.tile([128, 8, FSH], F32R, name="wv_sb", tag="wv")
                        nc.sync.dma_start(
                            out=wv_sb,
                            in_=wv[p][:].rearrange("p (e f) -> p e f", e=8),
                        )
                        for qb in range(nkb):
                            ps = psum_tile()
                            for e in range(8):
                                nc.tensor.matmul(
                                    ps[:, 0:FSH],
                                    xv_sb[:, e, qb * 128 : (qb + 1) * 128],
                                    wv_sb[:, e, :],
                                    start=(e == 0),
                                    stop=(e == 7),
                                )
                            nc.vector.tensor_copy(
                                vt[p][:, qb, :, 0:64],
                                ps[:, 0:FSH].rearrange("p (h d) -> p h d", h=HQ),
                            )

                # ---- phase B: attention (needs qt/kt => inside pqk) ----
                for path in (0, 1):
                    for pair in (0, 1):
                        hA, hB = 2 * pair, 2 * pair + 1
                        for qc in range(nqc):
                            plist = pieces_by_qc[qc]
                            if not plist:
                                continue
                            cA = psum_tile(tag="pCA")
                            cB = psum_tile(tag="pCB")
                            nkp = len(plist)
                            scale = 1.0 / math.sqrt(HEAD_DIM)
                            for ip, piece in enumerate(plist):
                                kb = piece["kb"]
                                qs = piece["qb0"] * KB
                                n = (piece["qb1"] - piece["qb0"] + 1) * KB
                                rel0 = qs - qc * QC
                                ksl = slice(kb * KB, (kb + 1) * KB)
                                qsl = slice(qs, qs + n)
                                # both heads' scores in one 2-bank psum tile
                                sAB = psum_tile(tag="pS")
                                nc.tensor.matmul(
                                    sAB[:, 0:n],
                                    kt[path][0:64, pair, ksl],
                                    qt[path][0:64, pair, qsl],
                                    start=True,
                                    stop=True,
                                )
                                nc.tensor.matmul(
                                    sAB[:, QC : QC + n],
                                    kt[path][64:128, pair, ksl],
                                    qt[path][64:128, pair, qsl],
                                    start=True,
                                    stop=True,
                                )
                                for qb, ti in piece["mixed"]:
                                    r = qb * KB - qs
                                    for off in (0, QC):
                                        msl = slice(off + r, off + r + KB)
                                        nc.vector.tensor_add(
                                            sAB[:, msl], sAB[:, msl], mt_sb[:, ti, :]
                                        )
                                eAB = wk_pool.tile(
                                    [128, 2 * QC], F32R, tag="eAB", bufs=4
                                )
                                if n == QC:
                                    nc.scalar.activation(
                                        eAB[:], sAB[:],
                                        mybir.ActivationFunctionType.Exp, scale=scale,
                                    )
                                else:
                                    nc.scalar.activation(
                                        eAB[:, 0:n], sAB[:, 0:n],
                                        mybir.ActivationFunctionType.Exp, scale=scale,
                                    )
                                    nc.scalar.activation(
                                        eAB[:, QC : QC + n], sAB[:, QC : QC + n],
                                        mybir.ActivationFunctionType.Exp, scale=scale,
                                    )
                                for h, off in ((hA, 0), (hB, QC)):
                                    dst = wout[path][
                                        h, piece["poff"] : piece["poff"] + 128 * n
                                    ].rearrange("(p q) -> p q", p=128)
                                    nc.sync.dma_start(
                                        out=dst, in_=eAB[:, off : off + n]
                                    )
                                nc.tensor.matmul(
                                    cA[0:65, rel0 : rel0 + n],
                                    vt[path][:, kb, hA, :],
                                    eAB[:, 0:n],
                                    start=(ip == 0),
                                    stop=(ip == nkp - 1),
                                )
                                nc.tensor.matmul(
                                    cB[0:65, rel0 : rel0 + n],
                                    vt[path][:, kb, hB, :],
                                    eAB[:, QC : QC + n],
                                    start=(ip == 0),
                                    stop=(ip == nkp - 1),
                                )
                            qsl = slice(qc * QC, (qc + 1) * QC)
                            if path == 0:
                                nc.vector.tensor_copy(
                                    ctx[pair][0:64, qsl], cA[0:64, :]
                                )
                                nc.vector.tensor_copy(
                                    ctx[pair][64:128, qsl], cB[0:64, :]
                                )
                            else:
                                nc.vector.tensor_mul(
                                    ctx[pair][0:64, qsl], cA[0:64, :],
                                    ctx[pair][0:64, qsl],
                                )
                                nc.vector.tensor_mul(
                                    ctx[pair][64:128, qsl], cB[0:64, :],
                                    ctx[pair][64:128, qsl],
                                )
                            nc.scalar.copy(
                                den_ab[path][32 * hA : 32 * hA + 1, qsl], cA[64:65, :]
                            )
                            nc.scalar.copy(
                                den_ab[path][32 * hB : 32 * hB + 1, qsl], cB[64:65, :]
                            )

            # ---- phase C: fuse, normalize, output projection ----
            with tc.tile_pool(name="px4", bufs=1) as px4:
                wo_sb = px4.tile([128, 2, N_EMBD], F32R, name="wo_sb")
                nc.sync.dma_start(out=wo_sb[:, 0, :], in_=wo[:, 0:N_EMBD])
                nc.sync.dma_start(out=wo_sb[:, 1, :], in_=wo[:, N_EMBD:])
                rr_t = [px4.tile([1, S], F32, name=f"rr{h}") for h in range(4)]
                rrc_t = [px4.tile([1, S], F32R, name=f"rrc{h}") for h in range(4)]
                for h in range(4):
                    row = slice(32 * h, 32 * h + 1)
                    nc.vector.tensor_mul(
                        rr_t[h][:], den_ab[0][row, :], den_ab[1][row, :]
                    )
                    with nc.allow_low_precision(reason="f32r is 32-bit storage"):
                        nc.vector.reciprocal(rrc_t[h][:], rr_t[h][:])
                for t in (0, 1):
                    for qc in range(nqc):
                        qsl = slice(qc * QC, (qc + 1) * QC)
                        aps = psum_tile()
                        nc.tensor.matmul(
                            aps[:],
                            sel_sb[:, 0:128],
                            rrc_t[2 * t][:, qsl],
                            start=True,
                            stop=False,
                        )
                        nc.tensor.matmul(
                            aps[:],
                            sel_sb[:, 128:256],
                            rrc_t[2 * t + 1][:, qsl],
                            start=False,
                            stop=True,
                        )
                        nc.vector.tensor_mul(ctx[t][:, qsl], ctx[t][:, qsl], aps[:])
                for qb in range(S // 128):
                    for fb2 in (0, 1):
                        ps = psum_tile()
                        for t in (0, 1):
                            nc.tensor.matmul(
                                ps[:],
                                ctx[t][:, qb * 128 : (qb + 1) * 128],
                                wo_sb[:, t, fb2 * QC : (fb2 + 1) * QC],
                                start=(t == 0),
                                stop=(t == 1),
                            )
                        ost = wk_pool.tile([128, QC], F32, tag="ost")
                        nc.scalar.copy(ost[:], ps[:])
                        nc.sync.dma_start(
                            out=outp[
                                qb * 128 : (qb + 1) * 128, fb2 * QC : (fb2 + 1) * QC
                            ],
                            in_=ost[:],
                        )
                for p in (0, 1):
                    for h in range(4):
                        nc.sync.dma_start(
                            out=dent[p * 4 + h : p * 4 + h + 1, :],
                            in_=den_ab[p][32 * h : 32 * h + 1, :],
                        )

    _legalize_waits(nc)
    return nc


# --------------------------------------------------------------------------
# host orchestration
# --------------------------------------------------------------------------
def _ensure_profile_hook():
    """Best-effort: make trace=True work under axon (test-time only)."""
    import sys
    import types

    try:
        from concourse import bass_utils as _bu

        if not getattr(_bu, "_safe_upload_installed", False):
            _orig = _bu.upload_artifacts

            def _safe_upload(tmpdir):
                try:
                    return _orig(tmpdir)
                except Exception:
                    return tmpdir

            _bu.upload_artifacts = _safe_upload
            _bu._safe_upload_installed = True
    except Exception:
        pass
    try:
        from antenv.axon_hooks import get_axon_ntff_profile_hook  # noqa: F401

        return
    except ImportError:
        pass
    try:
        import antenv

        mod = types.ModuleType("antenv.axon_hooks")
        _h = [None]
        mod.get_axon_ntff_profile_hook = lambda: _h[0]
        mod.set_axon_ntff_profile_hook = lambda hook: _h.__setitem__(0, hook)
        sys.modules["antenv.axon_hooks"] = mod
        antenv.axon_hooks = mod
        from trn_agent_boot.trn_boot import _ntff_profile_via_ctypes

        mod.set_axon_ntff_profile_hook(
            _ntff_profile_via_ctypes("/opt/axon/libaxon_pjrt.so")
        )
    except Exception:
        pass


def _interleave_et(arr_t, width):
    """[8*128, width] -> [128, 8*width] with [p, e*width + c] = arr_t[e*128+p, c]"""
    return np.ascontiguousarray(
        arr_t.reshape(8, 128, width).transpose(1, 0, 2).reshape(128, 8 * width)
    )


def _rope_tables(S):
    half = HEAD_DIM // 2
    inv_freq = 1.0 / (10000.0 ** (np.arange(half, dtype=np.float32) / half))
    ang = np.arange(S, dtype=np.float32)[None, :] * inv_freq[:, None]  # [32, S]
    c = np.cos(ang).astype(np.float32)
    s = np.sin(ang).astype(np.float32)
    C = np.tile(c, (4, 1))
    Sm = np.concatenate([-s, s, -s, s], axis=0)
    return C.astype(ml_dtypes.bfloat16), Sm.astype(ml_dtypes.bfloat16)


def kernel(query, key, value, mask, WQ1, WK1, WV1, WQ2, WK2, WV2, WO):
    global LAST_EXEC_NS
    query = np.asarray(query, dtype=np.float32)
    key_ = np.asarray(key, dtype=np.float32)
    value = np.asarray(value, dtype=np.float32)
    mask = np.asarray(mask)
    Ws = [np.asarray(w, dtype=np.float32) for w in (WQ1, WK1, WV1, WQ2, WK2, WV2, WO)]
    WQ1, WK1, WV1, WQ2, WK2, WV2, WO = Ws
    B, S, E = query.shape
    assert E == N_EMBD and B == 2 and S % QC == 0

    mask2d = np.broadcast_to(mask, (1, 1, S, S))[0, 0].astype(bool)
    kstat, mtiles = _classify_mask(mask2d)
    pieces_by_qc, npack = _plan_pieces(kstat, S)
    if _need_zero_tile(pieces_by_qc):
        mtiles = list(mtiles) + [np.full((128, 128), -1e9, np.float32)]
        zidx = len(mtiles) - 1
        for pl in pieces_by_qc:
            for p in pl:
                p["mixed"] = [(qb, ti if ti >= 0 else zidx) for qb, ti in p["mixed"]]
    n_mtiles = len(mtiles)

    key_sig = (S, n_mtiles, tuple(
        (p["kb"], p["qb0"], p["qb1"], tuple(p["mixed"]))
        for pl in pieces_by_qc for p in pl
    ))
    if key_sig not in _prog_cache:
        _prog_cache[key_sig] = _build_program(S, pieces_by_qc, npack, n_mtiles)
    nc = _prog_cache[key_sig]

    nkb = S // KB
    ropeC, ropeS = _rope_tables(S)
    sel = np.zeros((1, 256), np.float32)
    sel[0, 0:64] = 1.0
    sel[0, 192:256] = 1.0
    onesv = np.ones((128, nkb * HQ), np.float32)
    mt_flat = (
        np.ascontiguousarray(np.stack(mtiles, axis=1).reshape(128, n_mtiles * 128))
        if n_mtiles
        else None
    )

    in_maps = []
    for core in range(N_CORES):
        b, hq = core // 4, core % 4
        sh = slice(hq * FSH, (hq + 1) * FSH)
        im = {
            "xq": _interleave_et(query[b].T, S).astype(ml_dtypes.bfloat16),
            "xk": _interleave_et(key_[b].T, S).astype(ml_dtypes.bfloat16),
            "xv": _interleave_et(value[b].T, S),
            "wq1": _interleave_et(WQ1[sh].T, FSH).astype(ml_dtypes.bfloat16),
            "wq2": _interleave_et(WQ2[sh].T, FSH).astype(ml_dtypes.bfloat16),
            "wk1": _interleave_et(WK1[sh].T, FSH).astype(ml_dtypes.bfloat16),
            "wk2": _interleave_et(WK2[sh].T, FSH).astype(ml_dtypes.bfloat16),
            "wv1": _interleave_et(WV1[sh].T, FSH),
            "wv2": _interleave_et(WV2[sh].T, FSH),
            "wo": np.ascontiguousarray(
                WO[:, sh].T.reshape(2, 128, N_EMBD).transpose(1, 0, 2).reshape(128, -1)
            ),
            "ropec": ropeC,
            "ropes": ropeS,
            "sel": sel,
            "onesv": onesv,
        }
        if mt_flat is not None:
            im["mtiles"] = mt_flat
        in_maps.append(im)

    if PROFILE:
        _ensure_profile_hook()
    try:
        res = run_bass_kernel_spmd(nc, in_maps, list(range(N_CORES)), trace=PROFILE)
    except Exception:
        if not PROFILE:
            raise
        res = run_bass_kernel_spmd(nc, in_maps, list(range(N_CORES)), trace=False)
    if res.exec_time_ns is not None:
        LAST_EXEC_NS = res.exec_time_ns

    # ---- assemble ----
    out = np.zeros((B, S, N_EMBD), np.float32)
    w1 = np.zeros((B, N_HEAD, S, S), np.float32)
    w2 = np.zeros((B, N_HEAD, S, S), np.float32)
    all_pieces = [p for pl in pieces_by_qc for p in pl]
    for core in range(N_CORES):
        r = res.results[core]
        b, hq = core // 4, core % 4
        den = r["den"]  # [8, S]
        for path, (wp_name, wfull) in enumerate((("w1p", w1), ("w2p", w2))):
            wp = r[wp_name]
            for h in range(HQ):
                H = hq * HQ + h
                recip = 1.0 / den[path * 4 + h]  # [S]
                for p in all_pieces:
                    n = (p["qb1"] - p["qb0"] + 1) * KB
                    qs = p["qb0"] * KB
                    kb = p["kb"]
                    blockT = wp[h, p["poff"] : p["poff"] + 128 * n].reshape(128, n)
                    wfull[b, H, qs : qs + n, kb * KB : (kb + 1) * KB] = (
                        blockT.T * recip[qs : qs + n, None]
                    )
        out[b] += r["outp"]
    return out, w1, w2


# revision 22
# speedup vs baseline: 1.2882x; 1.2882x over previous
"""Dual multi-head attention (two attention paths, elementwise-fused) for
Trainium2, SPMD over 8 NeuronCores.

Sharding: core c -> batch b = c//4, head-quad hq = c%4 (4 of 16 heads).
Each core computes both attention paths for its (b, head-quad) shard:
  - Q/K projections in transposed layout [feat, seq] (bf16) with RoPE fused
    into the PSUM evacuation (swap-copy trick).
  - scores^T[k, q] per head via row-group-packed K=64 bf16 matmuls
    (two heads concurrently in the 128x128 PE array).
  - causal/any masking via additive -1e9 bias tiles on PSUM blocks that the
    host classifies as "mixed"; blocks that are fully masked are skipped
    entirely (device outputs are pre-zeroed by the runtime).
  - exp on ScalarE (scale=1/sqrt(d) folded in) -> unnormalized weights
    (float32r) which are both DMA'd out (packed) and fed to the ctx matmul.
  - ctx^T = [V | 1]^T @ expS accumulated over k-blocks (f32r, M=65; row 64
    gives the softmax denominator).
  - the two paths' unnormalized ctx are multiplied elementwise, scaled by
    1/(d1*d2) (broadcast via a tiny K=2 selector matmul), then projected
    through the WO column-shard. Host sums the 4 head-quad partials per batch.
  - softmax normalization of the exported weights happens on the host
    (w = expS^T / den), as does the [k,q] -> [q,k] transpose.
"""

import math
import os

import ml_dtypes
import numpy as np

import concourse.bass as bass
import concourse.mybir as mybir
import concourse.tile as tile
from concourse.bass_utils import run_bass_kernel_spmd

F32 = mybir.dt.float32
F32R = mybir.dt.float32r
BF16 = mybir.dt.bfloat16

N_HEAD = 16
N_EMBD = 1024
HEAD_DIM = 64
N_CORES = 8
HQ = 4  # heads per core
FSH = HQ * HEAD_DIM  # 256 features per core shard
QC = 512  # q chunk (columns per matmul)
KB = 128  # k block

PROFILE = False
LAST_EXEC_NS = None

_prog_cache = {}


# --------------------------------------------------------------------------
# wait legalization: CoreV3 ISA has a single sync-wait slot per instruction
# --------------------------------------------------------------------------
_waitfix_counter = [0]


def _legalize_waits(nc, limit=1):
    n_inserted = 0
    for bb in nc.main_func.blocks:
        insts = bb.instructions
        i = 0
        while i < len(insts):
            inst = insts[i]
            si = inst.sync_info
            if si is None or not si.on_wait:
                i += 1
                continue
            waits = list(si.on_wait)
            if len(waits) <= limit:
                i += 1
                continue
            excess, keep = waits[:-limit], waits[-limit:]
            nops = []
            for j in range(0, len(excess), limit):
                chunk = excess[j : j + limit]
                _waitfix_counter[0] += 1
                nop = mybir.InstNoOp(
                    name=f"I-waitfix-{_waitfix_counter[0]}",
                    engine=inst.engine,
                    sync_info=mybir.SyncInfo(on_wait=chunk, on_update=[]),
                )
                nc.register_instruction(nop)
                nops.append(nop)
            inst.sync_info = mybir.SyncInfo(on_wait=keep, on_update=list(si.on_update))
            for k, nop in enumerate(nops):
                insts.insert(i + k, nop)
            n_inserted += len(nops)
            i += len(nops) + 1
    return n_inserted


# --------------------------------------------------------------------------
# mask block classification and piece planning (host side)
# --------------------------------------------------------------------------
def _classify_mask(mask2d):
    """mask2d: [S, S] bool, mask2d[q, k]. Blocks at 128x128 granularity in
    (kb, qb) orientation. Returns kstat[kb][qb] in {0=skip,1=full,2+m=mixed}
    and the list of unique mixed bias tiles [128k, 128q] (fp32, (mT-1)*1e9)."""
    S = mask2d.shape[0]
    nb = S // KB
    kstat = [[0] * nb for _ in range(nb)]
    tiles = []
    tile_idx = {}
    for kb in range(nb):
        for qb in range(nb):
            blk = mask2d[qb * KB : (qb + 1) * KB, kb * KB : (kb + 1) * KB]
            if blk.all():
                kstat[kb][qb] = 1
            elif not blk.any():
                kstat[kb][qb] = 0
            else:
                bt = ((blk.T.astype(np.float32)) - 1.0) * 1e9
                key = bt.tobytes()
                if key not in tile_idx:
                    tile_idx[key] = len(tiles)
                    tiles.append(bt)
                kstat[kb][qb] = 2 + tile_idx[key]
    return kstat, tiles


def _plan_pieces(kstat, S):
    """Plan per-(qc) lists of pieces. A piece is one (kb, qc) score/exp/ctx
    unit covering a contiguous run of q-blocks. Ensures the first piece of
    every qc covers the union span of all later pieces (PSUM has_written
    safety). Returns (pieces_by_qc, npack, piece_offsets).

    piece = dict(kb, qb0, qb1, mixed=[(qb, tile_idx)], poff)
    """
    nqc = S // QC
    nb = S // KB
    pieces_by_qc = []
    poff = 0
    for qc in range(nqc):
        qb_lo, qb_hi = qc * 4, qc * 4 + 4
        plist = []
        for kb in range(nb):
            qbs = [qb for qb in range(qb_lo, qb_hi) if kstat[kb][qb] != 0]
            if not qbs:
                continue
            plist.append({"kb": kb, "qb0": min(qbs), "qb1": max(qbs)})
        if not plist:
            pieces_by_qc.append([])
            continue
        # union span; first piece must cover it
        u0 = min(p["qb0"] for p in plist)
        u1 = max(p["qb1"] for p in plist)
        plist[0]["qb0"], plist[0]["qb1"] = u0, u1
        for p in plist:
            mixed = []
            for qb in range(p["qb0"], p["qb1"] + 1):
                st = kstat[p["kb"]][qb]
                if st >= 2:
                    mixed.append((qb, st - 2))
                elif st == 0:
                    mixed.append((qb, -1))  # inside-run skip: zero it out
            p["mixed"] = mixed
            p["poff"] = poff
            poff += KB * (p["qb1"] - p["qb0"] + 1) * KB
        pieces_by_qc.append(plist)
    return pieces_by_qc, poff


def _need_zero_tile(pieces_by_qc):
    return any(
        ti == -1 for pl in pieces_by_qc for p in pl for (_, ti) in p["mixed"]
    )


# --------------------------------------------------------------------------
# device program
# --------------------------------------------------------------------------
def _build_program(S, pieces_by_qc, npack, n_mtiles):
    nqc = S // QC
    nkb = S // KB
    nc = bass.Bass()

    dp = nc.declare_dram_parameter
    xq = dp("xq", [128, 8 * S], BF16, isOutput=False)
    xk = dp("xk", [128, 8 * S], BF16, isOutput=False)
    xv = dp("xv", [128, 8 * S], F32R, isOutput=False)
    wq = [dp(f"wq{p}", [128, 8 * FSH], BF16, isOutput=False) for p in (1, 2)]
    wk = [dp(f"wk{p}", [128, 8 * FSH], BF16, isOutput=False) for p in (1, 2)]
    wv = [dp(f"wv{p}", [128, 8 * FSH], F32R, isOutput=False) for p in (1, 2)]
    wo = dp("wo", [128, 2 * N_EMBD], F32R, isOutput=False)
    ropec = dp("ropec", [128, S], BF16, isOutput=False)
    ropes = dp("ropes", [128, S], BF16, isOutput=False)
    sel = dp("sel", [1, 256], F32R, isOutput=False)
    onesv = dp("onesv", [128, nkb * HQ], F32R, isOutput=False)
    if n_mtiles:
        mtin = dp("mtiles", [128, n_mtiles * 128], F32, isOutput=False)
    wout = [dp(f"w{p}p", [HQ, npack], F32R, isOutput=True) for p in (1, 2)]
    outp = dp("outp", [S, N_EMBD], F32, isOutput=True)
    dent = dp("den", [8, S], F32, isOutput=True)

    with tile.TileContext(nc) as tc:
        with (
            tc.tile_pool(name="static", bufs=1) as st,
            tc.tile_pool(name="work", bufs=3) as wk_pool,
            tc.tile_pool(name="psum", bufs=2, space="PSUM") as pp,
        ):
            # ---- static tiles (live across all phases) ----
            vt = [st.tile([128, nkb, HQ, 65], F32R, name=f"vt{p}") for p in (0, 1)]
            ctx = [st.tile([128, S], F32R, name=f"ctx{t}") for t in (0, 1)]
            # den rows at 32-aligned partitions: den_ab[path][32*h] = denom
            den_ab = [st.tile([128, S], F32, name=f"den{p}") for p in (0, 1)]
            sel_sb = st.tile([1, 256], F32R, name="sel_sb")
            if n_mtiles:
                mt_sb = st.tile([128, n_mtiles, 128], F32, name="mt_sb")

            nc.sync.dma_start(out=sel_sb, in_=sel[:])
            if n_mtiles:
                nc.sync.dma_start(
                    out=mt_sb,
                    in_=mtin[:].rearrange("p (m c) -> p m c", m=n_mtiles),
                )
            # ones columns of V tiles (col 64 of each [128, 65] group)
            for p in (0, 1):
                nc.sync.dma_start(
                    out=vt[p][:, :, :, 64:65],
                    in_=onesv[:].rearrange("p (k h) -> p k h", k=nkb)[:, :, :, None],
                )

            psum_ctr = [0]
            _ctx_tags = ("pC0A", "pC0B", "pC1A", "pC1B")

            def psum_tile(tag=None):
                # pS: [128,1024] double-bank (score pairs, bufs=2 -> 4 banks);
                # pC**: [128,512] single-bank accumulators (bufs=1 x4).
                if tag is None:
                    tag = _ctx_tags[psum_ctr[0] % 4]
                    psum_ctr[0] += 1
                if tag == "pS":
                    return pp.tile([128, 2 * QC], F32, tag=tag, name=tag, bufs=2)
                return pp.tile([128, QC], F32, tag=tag, name=tag, bufs=1)

            # ---- phase A: projections ----
            def proj_qk(x_sb, w_sb, out_tile, rc_sb, rs_sb):
                for fb in range(2):
                    for qc in range(nqc):
                        ps = psum_tile()
                        for e in range(8):
                            nc.tensor.matmul(
                                ps[:],
                                w_sb[:, e, fb * 128 : (fb + 1) * 128],
                                x_sb[:, e, qc * QC : (qc + 1) * QC],
                                start=(e == 0),
                                stop=(e == 7),
                            )
                        tmp = wk_pool.tile([128, QC], BF16, tag="tmp")
                        nc.scalar.copy(tmp[:], ps[:])
                        tsw = wk_pool.tile([128, QC], BF16, tag="tsw")
                        for blk in range(4):
                            src = blk ^ 1
                            nc.vector.tensor_copy(
                                tsw[blk * 32 : (blk + 1) * 32, :],
                                tmp[src * 32 : (src + 1) * 32, :],
                            )
                        qsl = slice(qc * QC, (qc + 1) * QC)
                        t1 = wk_pool.tile([128, QC], BF16, tag="t1")
                        nc.vector.tensor_mul(t1[:], tmp[:], rc_sb[:, qsl])
                        t2 = wk_pool.tile([128, QC], BF16, tag="t2")
                        nc.vector.tensor_mul(t2[:], tsw[:], rs_sb[:, qsl])
                        nc.vector.tensor_add(out_tile[:, fb, qsl], t1[:], t2[:])

            with tc.tile_pool(name="pqk", bufs=1) as pqk:
                qt = [pqk.tile([128, 2, S], BF16, name=f"qt{p}") for p in (0, 1)]
                kt = [pqk.tile([128, 2, S], BF16, name=f"kt{p}") for p in (0, 1)]

                for which, x_in, w_in, dst in (
                    ("q", xq, wq, qt),
                    ("k", xk, wk, kt),
                ):
                    with tc.tile_pool(name=f"px_{which}", bufs=1) as px:
                        x_sb = px.tile([128, 8, S], BF16, name=f"x{which}_sb")
                        nc.sync.dma_start(
                            out=x_sb, in_=x_in[:].rearrange("p (e q) -> p e q", e=8)
                        )
                        rc_sb = px.tile([128, S], BF16, name=f"rc_{which}")
                        rs_sb = px.tile([128, S], BF16, name=f"rs_{which}")
                        nc.sync.dma_start(out=rc_sb, in_=ropec[:])
                        nc.sync.dma_start(out=rs_sb, in_=ropes[:])
                        for p in (0, 1):
                            w_sb = px.tile(
                                [128, 8, FSH], BF16, name=f"w{which}_sb", tag="wqk"
                            )
                            nc.sync.dma_start(
                                out=w_sb,
                                in_=w_in[p][:].rearrange("p (e f) -> p e f", e=8),
                            )
                            proj_qk(x_sb, w_sb, dst[p], rc_sb, rs_sb)

                with tc.tile_pool(name="px_v", bufs=1) as px:
                    xv_sb = px.tile([128, 8, S], F32R, name="xv_sb")
                    nc.sync.dma_start(
                        out=xv_sb, in_=xv[:].rearrange("p (e q) -> p e q", e=8)
                    )
                    for p in (0, 1):
                        wv_sb = px.tile([128, 8, FSH], F32R, name="wv_sb", tag="wv")
                        nc.sync.dma_start(
                            out=wv_sb,
                            in_=wv[p][:].rearrange("p (e f) -> p e f", e=8),
                        )
                        for qb in range(nkb):
                            ps = psum_tile()
                            for e in range(8):
                                nc.tensor.matmul(
                                    ps[:, 0:FSH],
                                    xv_sb[:, e, qb * 128 : (qb + 1) * 128],
                                    wv_sb[:, e, :],
                                    start=(e == 0),
                                    stop=(e == 7),
                                )
                            nc.vector.tensor_copy(
                                vt[p][:, qb, :, 0:64],
                                ps[:, 0:FSH].rearrange("p (h d) -> p h d", h=HQ),
                            )

                # ---- phase B: attention (needs qt/kt => inside pqk) ----
                # Both paths interleaved piece-by-piece: while one path's exp
                # runs on ScalarE, the PE computes the other path's scores.
                scale = 1.0 / math.sqrt(HEAD_DIM)
                for pair in (0, 1):
                    hA, hB = 2 * pair, 2 * pair + 1
                    for qc in range(nqc):
                        plist = pieces_by_qc[qc]
                        if not plist:
                            continue
                        cacc = {
                            (0, "A"): psum_tile(tag="pC0A"),
                            (0, "B"): psum_tile(tag="pC0B"),
                            (1, "A"): psum_tile(tag="pC1A"),
                            (1, "B"): psum_tile(tag="pC1B"),
                        }
                        nkp = len(plist)
                        for ip, piece in enumerate(plist):
                            kb = piece["kb"]
                            qs = piece["qb0"] * KB
                            n = (piece["qb1"] - piece["qb0"] + 1) * KB
                            rel0 = qs - qc * QC
                            ksl = slice(kb * KB, (kb + 1) * KB)
                            qsl = slice(qs, qs + n)
                            for path in (0, 1):
                                # both heads' scores in one 2-bank psum tile
                                sAB = psum_tile(tag="pS")
                                nc.tensor.matmul(
                                    sAB[:, 0:n],
                                    kt[path][0:64, pair, ksl],
                                    qt[path][0:64, pair, qsl],
                                    start=True,
                                    stop=True,
                                )
                                nc.tensor.matmul(
                                    sAB[:, QC : QC + n],
                                    kt[path][64:128, pair, ksl],
                                    qt[path][64:128, pair, qsl],
                                    start=True,
                                    stop=True,
                                )
                                for qb, ti in piece["mixed"]:
                                    r = qb * KB - qs
                                    for off in (0, QC):
                                        msl = slice(off + r, off + r + KB)
                                        nc.vector.tensor_add(
                                            sAB[:, msl], sAB[:, msl],
                                            mt_sb[:, ti, :],
                                        )
                                eAB = wk_pool.tile(
                                    [128, 2 * QC], F32R, tag="eAB", bufs=4
                                )
                                if n == QC:
                                    nc.scalar.activation(
                                        eAB[:], sAB[:],
                                        mybir.ActivationFunctionType.Exp,
                                        scale=scale,
                                    )
                                else:
                                    nc.scalar.activation(
                                        eAB[:, 0:n], sAB[:, 0:n],
                                        mybir.ActivationFunctionType.Exp,
                                        scale=scale,
                                    )
                                    nc.scalar.activation(
                                        eAB[:, QC : QC + n], sAB[:, QC : QC + n],
                                        mybir.ActivationFunctionType.Exp,
                                        scale=scale,
                                    )
                                for h, off in ((hA, 0), (hB, QC)):
                                    dst = wout[path][
                                        h, piece["poff"] : piece["poff"] + 128 * n
                                    ].rearrange("(p q) -> p q", p=128)
                                    nc.sync.dma_start(
                                        out=dst, in_=eAB[:, off : off + n]
                                    )
                                nc.tensor.matmul(
                                    cacc[(path, "A")][0:65, rel0 : rel0 + n],
                                    vt[path][:, kb, hA, :],
                                    eAB[:, 0:n],
                                    start=(ip == 0),
                                    stop=(ip == nkp - 1),
                                )
                                nc.tensor.matmul(
                                    cacc[(path, "B")][0:65, rel0 : rel0 + n],
                                    vt[path][:, kb, hB, :],
                                    eAB[:, QC : QC + n],
                                    start=(ip == 0),
                                    stop=(ip == nkp - 1),
                                )
                        qsl = slice(qc * QC, (qc + 1) * QC)
                        for half, rows in (("A", slice(0, 64)), ("B", slice(64, 128))):
                            c0 = cacc[(0, half)]
                            c1 = cacc[(1, half)]
                            nc.vector.tensor_copy(ctx[pair][rows, qsl], c0[0:64, :])
                            nc.vector.tensor_mul(
                                ctx[pair][rows, qsl], c1[0:64, :],
                                ctx[pair][rows, qsl],
                            )
                            h = hA if half == "A" else hB
                            nc.scalar.copy(
                                den_ab[0][32 * h : 32 * h + 1, qsl], c0[64:65, :]
                            )
                            nc.scalar.copy(
                                den_ab[1][32 * h : 32 * h + 1, qsl], c1[64:65, :]
                            )

            # ---- phase C: fuse, normalize, output projection ----
            with tc.tile_pool(name="px4", bufs=1) as px4:
                wo_sb = px4.tile([128, 2, N_EMBD], F32R, name="wo_sb")
                nc.sync.dma_start(out=wo_sb[:, 0, :], in_=wo[:, 0:N_EMBD])
                nc.sync.dma_start(out=wo_sb[:, 1, :], in_=wo[:, N_EMBD:])
                rr_t = [px4.tile([1, S], F32, name=f"rr{h}") for h in range(4)]
                rrc_t = [px4.tile([1, S], F32R, name=f"rrc{h}") for h in range(4)]
                for h in range(4):
                    row = slice(32 * h, 32 * h + 1)
                    nc.vector.tensor_mul(
                        rr_t[h][:], den_ab[0][row, :], den_ab[1][row, :]
                    )
                    with nc.allow_low_precision(reason="f32r is 32-bit storage"):
                        nc.vector.reciprocal(rrc_t[h][:], rr_t[h][:])
                for t in (0, 1):
                    for qc in range(nqc):
                        qsl = slice(qc * QC, (qc + 1) * QC)
                        aps = psum_tile()
                        nc.tensor.matmul(
                            aps[:],
                            sel_sb[:, 0:128],
                            rrc_t[2 * t][:, qsl],
                            start=True,
                            stop=False,
                        )
                        nc.tensor.matmul(
                            aps[:],
                            sel_sb[:, 128:256],
                            rrc_t[2 * t + 1][:, qsl],
                            start=False,
                            stop=True,
                        )
                        nc.vector.tensor_mul(ctx[t][:, qsl], ctx[t][:, qsl], aps[:])
                for qb in range(S // 128):
                    for fb2 in (0, 1):
                        ps = psum_tile()
                        for t in (0, 1):
                            nc.tensor.matmul(
                                ps[:],
                                ctx[t][:, qb * 128 : (qb + 1) * 128],
                                wo_sb[:, t, fb2 * QC : (fb2 + 1) * QC],
                                start=(t == 0),
                                stop=(t == 1),
                            )
                        ost = wk_pool.tile([128, QC], F32, tag="ost")
                        nc.scalar.copy(ost[:], ps[:])
                        nc.sync.dma_start(
                            out=outp[
                                qb * 128 : (qb + 1) * 128, fb2 * QC : (fb2 + 1) * QC
                            ],
                            in_=ost[:],
                        )
                for p in (0, 1):
                    for h in range(4):
                        nc.sync.dma_start(
                            out=dent[p * 4 + h : p * 4 + h + 1, :],
                            in_=den_ab[p][32 * h : 32 * h + 1, :],
                        )

    _legalize_waits(nc)
    return nc


# --------------------------------------------------------------------------
# host orchestration
# --------------------------------------------------------------------------
def _ensure_profile_hook():
    """Best-effort: make trace=True work under axon (test-time only)."""
    import sys
    import types

    try:
        from concourse import bass_utils as _bu

        if not getattr(_bu, "_safe_upload_installed", False):
            _orig = _bu.upload_artifacts

            def _safe_upload(tmpdir):
                try:
                    return _orig(tmpdir)
                except Exception:
                    return tmpdir

            _bu.upload_artifacts = _safe_upload
            _bu._safe_upload_installed = True
    except Exception:
        pass
    try:
        from antenv.axon_hooks import get_axon_ntff_profile_hook  # noqa: F401

        return
    except ImportError:
        pass
    try:
        import antenv

        mod = types.ModuleType("antenv.axon_hooks")
        _h = [None]
        mod.get_axon_ntff_profile_hook = lambda: _h[0]
        mod.set_axon_ntff_profile_hook = lambda hook: _h.__setitem__(0, hook)
        sys.modules["antenv.axon_hooks"] = mod
        antenv.axon_hooks = mod
        from trn_agent_boot.trn_boot import _ntff_profile_via_ctypes

        mod.set_axon_ntff_profile_hook(
            _ntff_profile_via_ctypes("/opt/axon/libaxon_pjrt.so")
        )
    except Exception:
        pass


def _interleave_et(arr_t, width):
    """[8*128, width] -> [128, 8*width] with [p, e*width + c] = arr_t[e*128+p, c]"""
    return np.ascontiguousarray(
        arr_t.reshape(8, 128, width).transpose(1, 0, 2).reshape(128, 8 * width)
    )


def _rope_tables(S):
    half = HEAD_DIM // 2
    inv_freq = 1.0 / (10000.0 ** (np.arange(half, dtype=np.float32) / half))
    ang = np.arange(S, dtype=np.float32)[None, :] * inv_freq[:, None]  # [32, S]
    c = np.cos(ang).astype(np.float32)
    s = np.sin(ang).astype(np.float32)
    C = np.tile(c, (4, 1))
    Sm = np.concatenate([-s, s, -s, s], axis=0)
    return C.astype(ml_dtypes.bfloat16), Sm.astype(ml_dtypes.bfloat16)


def kernel(query, key, value, mask, WQ1, WK1, WV1, WQ2, WK2, WV2, WO):
    global LAST_EXEC_NS
    query = np.asarray(query, dtype=np.float32)
    key_ = np.asarray(key, dtype=np.float32)
    value = np.asarray(value, dtype=np.float32)
    mask = np.asarray(mask)
    Ws = [np.asarray(w, dtype=np.float32) for w in (WQ1, WK1, WV1, WQ2, WK2, WV2, WO)]
    WQ1, WK1, WV1, WQ2, WK2, WV2, WO = Ws
    B, S, E = query.shape
    assert E == N_EMBD and B == 2 and S % QC == 0

    mask2d = np.broadcast_to(mask, (1, 1, S, S))[0, 0].astype(bool)
    kstat, mtiles = _classify_mask(mask2d)
    pieces_by_qc, npack = _plan_pieces(kstat, S)
    if _need_zero_tile(pieces_by_qc):
        mtiles = list(mtiles) + [np.full((128, 128), -1e9, np.float32)]
        zidx = len(mtiles) - 1
        for pl in pieces_by_qc:
            for p in pl:
                p["mixed"] = [(qb, ti if ti >= 0 else zidx) for qb, ti in p["mixed"]]
    n_mtiles = len(mtiles)

    key_sig = (S, n_mtiles, tuple(
        (p["kb"], p["qb0"], p["qb1"], tuple(p["mixed"]))
        for pl in pieces_by_qc for p in pl
    ))
    if key_sig not in _prog_cache:
        _prog_cache[key_sig] = _build_program(S, pieces_by_qc, npack, n_mtiles)
    nc = _prog_cache[key_sig]

    nkb = S // KB
    ropeC, ropeS = _rope_tables(S)
    sel = np.zeros((1, 256), np.float32)
    sel[0, 0:64] = 1.0
    sel[0, 192:256] = 1.0
    onesv = np.ones((128, nkb * HQ), np.float32)
    mt_flat = (
        np.ascontiguousarray(np.stack(mtiles, axis=1).reshape(128, n_mtiles * 128))
        if n_mtiles
        else None
    )

    in_maps = []
    for core in range(N_CORES):
        b, hq = core // 4, core % 4
        sh = slice(hq * FSH, (hq + 1) * FSH)
        im = {
            "xq": _interleave_et(query[b].T, S).astype(ml_dtypes.bfloat16),
            "xk": _interleave_et(key_[b].T, S).astype(ml_dtypes.bfloat16),
            "xv": _interleave_et(value[b].T, S),
            "wq1": _interleave_et(WQ1[sh].T, FSH).astype(ml_dtypes.bfloat16),
            "wq2": _interleave_et(WQ2[sh].T, FSH).astype(ml_dtypes.bfloat16),
            "wk1": _interleave_et(WK1[sh].T, FSH).astype(ml_dtypes.bfloat16),
            "wk2": _interleave_et(WK2[sh].T, FSH).astype(ml_dtypes.bfloat16),
            "wv1": _interleave_et(WV1[sh].T, FSH),
            "wv2": _interleave_et(WV2[sh].T, FSH),
            "wo": np.ascontiguousarray(
                WO[:, sh].T.reshape(2, 128, N_EMBD).transpose(1, 0, 2).reshape(128, -1)
            ),
            "ropec": ropeC,
            "ropes": ropeS,
            "sel": sel,
            "onesv": onesv,
        }
        if mt_flat is not None:
            im["mtiles"] = mt_flat
        in_maps.append(im)

    if PROFILE:
        _ensure_profile_hook()
    try:
        res = run_bass_kernel_spmd(nc, in_maps, list(range(N_CORES)), trace=PROFILE)
    except Exception:
        if not PROFILE:
            raise
        res = run_bass_kernel_spmd(nc, in_maps, list(range(N_CORES)), trace=False)
    if res.exec_time_ns is not None:
        LAST_EXEC_NS = res.exec_time_ns

    # ---- assemble ----
    out = np.zeros((B, S, N_EMBD), np.float32)
    w1 = np.zeros((B, N_HEAD, S, S), np.float32)
    w2 = np.zeros((B, N_HEAD, S, S), np.float32)
    all_pieces = [p for pl in pieces_by_qc for p in pl]
    for core in range(N_CORES):
        r = res.results[core]
        b, hq = core // 4, core % 4
        den = r["den"]  # [8, S]
        for path, (wp_name, wfull) in enumerate((("w1p", w1), ("w2p", w2))):
            wp = r[wp_name]
            for h in range(HQ):
                H = hq * HQ + h
                recip = 1.0 / den[path * 4 + h]  # [S]
                for p in all_pieces:
                    n = (p["qb1"] - p["qb0"] + 1) * KB
                    qs = p["qb0"] * KB
                    kb = p["kb"]
                    blockT = wp[h, p["poff"] : p["poff"] + 128 * n].reshape(128, n)
                    wfull[b, H, qs : qs + n, kb * KB : (kb + 1) * KB] = (
                        blockT.T * recip[qs : qs + n, None]
                    )
        out[b] += r["outp"]
    return out, w1, w2


# revision 27
# speedup vs baseline: 1.5934x; 1.2369x over previous
"""Dual multi-head attention (two attention paths, elementwise-fused) for
Trainium2, SPMD over 8 NeuronCores.

Sharding: core c -> batch b = c//4, head-quad hq = c%4 (4 of 16 heads).
Each core computes both attention paths for its (b, head-quad) shard:
  - Q/K projections in transposed layout [feat, seq] (bf16) with RoPE fused
    into the PSUM evacuation (swap-copy trick).
  - scores^T[k, q] per head via row-group-packed K=64 bf16 matmuls
    (two heads concurrently in the 128x128 PE array).
  - causal/any masking via additive -1e9 bias tiles on PSUM blocks that the
    host classifies as "mixed"; blocks that are fully masked are skipped
    entirely (device outputs are pre-zeroed by the runtime).
  - exp on ScalarE (scale=1/sqrt(d) folded in) -> unnormalized weights
    (float32r) which are both DMA'd out (packed) and fed to the ctx matmul.
  - ctx^T = [V | 1]^T @ expS accumulated over k-blocks (f32r, M=65; row 64
    gives the softmax denominator).
  - the two paths' unnormalized ctx are multiplied elementwise, scaled by
    1/(d1*d2) (broadcast via a tiny K=2 selector matmul), then projected
    through the WO column-shard. Host sums the 4 head-quad partials per batch.
  - softmax normalization of the exported weights happens on the host
    (w = expS^T / den), as does the [k,q] -> [q,k] transpose.
"""

import math
import os

import ml_dtypes
import numpy as np

import concourse.bass as bass
import concourse.mybir as mybir
import concourse.tile as tile
from concourse.bass_utils import run_bass_kernel_spmd

F32 = mybir.dt.float32
F32R = mybir.dt.float32r
BF16 = mybir.dt.bfloat16

N_HEAD = 16
N_EMBD = 1024
HEAD_DIM = 64
N_CORES = 8
HQ = 4  # heads per core
FSH = HQ * HEAD_DIM  # 256 features per core shard
QC = 512  # q chunk (columns per matmul)
KB = 128  # k block

PROFILE = False
LAST_EXEC_NS = None

_prog_cache = {}


# --------------------------------------------------------------------------
# wait legalization: CoreV3 ISA has a single sync-wait slot per instruction
# --------------------------------------------------------------------------
_waitfix_counter = [0]


def _legalize_waits(nc, limit=1):
    n_inserted = 0
    for bb in nc.main_func.blocks:
        insts = bb.instructions
        i = 0
        while i < len(insts):
            inst = insts[i]
            si = inst.sync_info
            if si is None or not si.on_wait:
                i += 1
                continue
            waits = list(si.on_wait)
            if len(waits) <= limit:
                i += 1
                continue
            excess, keep = waits[:-limit], waits[-limit:]
            nops = []
            for j in range(0, len(excess), limit):
                chunk = excess[j : j + limit]
                _waitfix_counter[0] += 1
                nop = mybir.InstNoOp(
                    name=f"I-waitfix-{_waitfix_counter[0]}",
                    engine=inst.engine,
                    sync_info=mybir.SyncInfo(on_wait=chunk, on_update=[]),
                )
                nc.register_instruction(nop)
                nops.append(nop)
            inst.sync_info = mybir.SyncInfo(on_wait=keep, on_update=list(si.on_update))
            for k, nop in enumerate(nops):
                insts.insert(i + k, nop)
            n_inserted += len(nops)
            i += len(nops) + 1
    return n_inserted


# --------------------------------------------------------------------------
# mask block classification and piece planning (host side)
# --------------------------------------------------------------------------
def _classify_mask(mask2d):
    """mask2d: [S, S] bool, mask2d[q, k]. Blocks at 128x128 granularity in
    (kb, qb) orientation. Returns kstat[kb][qb] in {0=skip,1=full,2+m=mixed}
    and the list of unique mixed bias tiles [128k, 128q] (fp32, (mT-1)*1e9)."""
    S = mask2d.shape[0]
    nb = S // KB
    kstat = [[0] * nb for _ in range(nb)]
    tiles = []
    tile_idx = {}
    for kb in range(nb):
        for qb in range(nb):
            blk = mask2d[qb * KB : (qb + 1) * KB, kb * KB : (kb + 1) * KB]
            if blk.all():
                kstat[kb][qb] = 1
            elif not blk.any():
                kstat[kb][qb] = 0
            else:
                bt = ((blk.T.astype(np.float32)) - 1.0) * 1e9
                key = bt.tobytes()
                if key not in tile_idx:
                    tile_idx[key] = len(tiles)
                    tiles.append(bt)
                kstat[kb][qb] = 2 + tile_idx[key]
    return kstat, tiles


def _plan_pieces(kstat, S):
    """Plan per-(qc) lists of pieces. A piece is one (kb, qc) score/exp/ctx
    unit covering a contiguous run of q-blocks. Ensures the first piece of
    every qc covers the union span of all later pieces (PSUM has_written
    safety). Returns (pieces_by_qc, npack, piece_offsets).

    piece = dict(kb, qb0, qb1, mixed=[(qb, tile_idx)], poff)
    """
    nqc = S // QC
    nb = S // KB
    pieces_by_qc = []
    poff = 0
    for qc in range(nqc):
        qb_lo, qb_hi = qc * 4, qc * 4 + 4
        plist = []
        for kb in range(nb):
            qbs = [qb for qb in range(qb_lo, qb_hi) if kstat[kb][qb] != 0]
            if not qbs:
                continue
            plist.append({"kb": kb, "qb0": min(qbs), "qb1": max(qbs)})
        if not plist:
            pieces_by_qc.append([])
            continue
        # union span; first piece must cover it
        u0 = min(p["qb0"] for p in plist)
        u1 = max(p["qb1"] for p in plist)
        plist[0]["qb0"], plist[0]["qb1"] = u0, u1
        for p in plist:
            mixed = []
            for qb in range(p["qb0"], p["qb1"] + 1):
                st = kstat[p["kb"]][qb]
                if st >= 2:
                    mixed.append((qb, st - 2))
                elif st == 0:
                    mixed.append((qb, -1))  # inside-run skip: zero it out
            p["mixed"] = mixed
            p["poff"] = poff
            poff += KB * (p["qb1"] - p["qb0"] + 1) * KB
        pieces_by_qc.append(plist)
    return pieces_by_qc, poff


def _need_zero_tile(pieces_by_qc):
    return any(
        ti == -1 for pl in pieces_by_qc for p in pl for (_, ti) in p["mixed"]
    )


# --------------------------------------------------------------------------
# device program
# --------------------------------------------------------------------------
def _build_program(S, pieces_by_qc, npack, n_mtiles):
    nqc = S // QC
    nkb = S // KB
    nc = bass.Bass()

    dp = nc.declare_dram_parameter
    xq = dp("xq", [128, 8 * S], BF16, isOutput=False)
    xk = dp("xk", [128, 8 * S], BF16, isOutput=False)
    xv = dp("xv", [128, 8 * S], BF16, isOutput=False)
    wq = [dp(f"wq{p}", [128, 8 * FSH], BF16, isOutput=False) for p in (1, 2)]
    wk = [dp(f"wk{p}", [128, 8 * FSH], BF16, isOutput=False) for p in (1, 2)]
    wv = [dp(f"wv{p}", [128, 8 * FSH], BF16, isOutput=False) for p in (1, 2)]
    wo = dp("wo", [128, 2 * N_EMBD], F32R, isOutput=False)
    ropec = dp("ropec", [128, S], BF16, isOutput=False)
    ropes = dp("ropes", [128, S], BF16, isOutput=False)
    sel = dp("sel", [1, 256], F32R, isOutput=False)
    onesv = dp("onesv", [128, nkb * HQ], BF16, isOutput=False)
    if n_mtiles:
        mtin = dp("mtiles", [128, n_mtiles * 128], F32, isOutput=False)
    wout = [dp(f"w{p}p", [HQ, npack], BF16, isOutput=True) for p in (1, 2)]
    outp = dp("outp", [S, N_EMBD], F32, isOutput=True)
    dent = dp("den", [8, S], F32, isOutput=True)

    with tile.TileContext(nc) as tc:
        with (
            tc.tile_pool(name="static", bufs=1) as st,
            tc.tile_pool(name="work", bufs=3) as wk_pool,
            tc.tile_pool(name="psum", bufs=2, space="PSUM") as pp,
        ):
            # ---- static tiles (live across all phases) ----
            vt = [st.tile([128, nkb, HQ, 65], BF16, name=f"vt{p}") for p in (0, 1)]
            ctx = [st.tile([128, S], F32R, name=f"ctx{t}") for t in (0, 1)]
            # den rows at 32-aligned partitions: den_ab[path][32*h] = denom
            den_ab = [st.tile([128, S], F32, name=f"den{p}") for p in (0, 1)]
            sel_sb = st.tile([1, 256], F32R, name="sel_sb")
            if n_mtiles:
                mt_sb = st.tile([128, n_mtiles, 128], F32, name="mt_sb")

            nc.sync.dma_start(out=sel_sb, in_=sel[:])
            if n_mtiles:
                nc.sync.dma_start(
                    out=mt_sb,
                    in_=mtin[:].rearrange("p (m c) -> p m c", m=n_mtiles),
                )
            # ones columns of V tiles (col 64 of each [128, 65] group)
            for p in (0, 1):
                nc.sync.dma_start(
                    out=vt[p][:, :, :, 64:65],
                    in_=onesv[:].rearrange("p (k h) -> p k h", k=nkb)[:, :, :, None],
                )

            psum_ctr = [0]
            _ctx_tags = ("pC0A", "pC0B", "pC1A", "pC1B")

            def psum_tile(tag=None):
                # pS: [128,1024] double-bank (score pairs, bufs=2 -> 4 banks);
                # pC**: [128,512] single-bank accumulators (bufs=1 x4).
                if tag is None:
                    tag = _ctx_tags[psum_ctr[0] % 4]
                    psum_ctr[0] += 1
                if tag == "pS":
                    return pp.tile([128, 2 * QC], F32, tag=tag, name=tag, bufs=2)
                return pp.tile([128, QC], F32, tag=tag, name=tag, bufs=1)

            # ---- phase A: projections ----
            def proj_qk(x_sb, w_sb, out_tile, rc_sb, rs_sb):
                for fb in range(2):
                    for qc in range(nqc):
                        ps = psum_tile()
                        for e in range(8):
                            nc.tensor.matmul(
                                ps[:],
                                w_sb[:, e, fb * 128 : (fb + 1) * 128],
                                x_sb[:, e, qc * QC : (qc + 1) * QC],
                                start=(e == 0),
                                stop=(e == 7),
                            )
                        tmp = wk_pool.tile([128, QC], BF16, tag="tmp")
                        nc.scalar.copy(tmp[:], ps[:])
                        tsw = wk_pool.tile([128, QC], BF16, tag="tsw")
                        for blk in range(4):
                            src = blk ^ 1
                            nc.vector.tensor_copy(
                                tsw[blk * 32 : (blk + 1) * 32, :],
                                tmp[src * 32 : (src + 1) * 32, :],
                            )
                        qsl = slice(qc * QC, (qc + 1) * QC)
                        t1 = wk_pool.tile([128, QC], BF16, tag="t1")
                        nc.vector.tensor_mul(t1[:], tmp[:], rc_sb[:, qsl])
                        t2 = wk_pool.tile([128, QC], BF16, tag="t2")
                        nc.vector.tensor_mul(t2[:], tsw[:], rs_sb[:, qsl])
                        nc.vector.tensor_add(out_tile[:, fb, qsl], t1[:], t2[:])

            with tc.tile_pool(name="pqk", bufs=1) as pqk:
                qt = [pqk.tile([128, 2, S], BF16, name=f"qt{p}") for p in (0, 1)]
                kt = [pqk.tile([128, 2, S], BF16, name=f"kt{p}") for p in (0, 1)]
                rc_sb = pqk.tile([128, S], BF16, name="rc_sb")
                rs_sb = pqk.tile([128, S], BF16, name="rs_sb")
                nc.sync.dma_start(out=rc_sb, in_=ropec[:])
                nc.sync.dma_start(out=rs_sb, in_=ropes[:])

                def load_w(pw, w_in):
                    w_sb = pw.tile([128, 8, FSH], BF16, name="w_sb", tag="w", bufs=2)
                    nc.sync.dma_start(
                        out=w_sb, in_=w_in[:].rearrange("p (e f) -> p e f", e=8)
                    )
                    return w_sb

                # px_k opened first so xk's DMA prefetches during the Q phase;
                # px_v reuses px_q's space, so xv prefetches during the K phase
                with tc.tile_pool(name="pw", bufs=1) as pw:
                    with tc.tile_pool(name="px_k", bufs=1) as pxk:
                        xk_sb = pxk.tile([128, 8, S], BF16, name="xk_sb")
                        nc.sync.dma_start(
                            out=xk_sb, in_=xk[:].rearrange("p (e q) -> p e q", e=8)
                        )
                        with tc.tile_pool(name="px_q", bufs=1) as pxq:
                            xq_sb = pxq.tile([128, 8, S], BF16, name="xq_sb")
                            nc.sync.dma_start(
                                out=xq_sb,
                                in_=xq[:].rearrange("p (e q) -> p e q", e=8),
                            )
                            for p in (0, 1):
                                proj_qk(xq_sb, load_w(pw, wq[p]), qt[p], rc_sb, rs_sb)
                        with tc.tile_pool(name="px_v", bufs=1) as pxv:
                            xv_sb = pxv.tile([128, 8, S], BF16, name="xv_sb")
                            nc.sync.dma_start(
                                out=xv_sb,
                                in_=xv[:].rearrange("p (e q) -> p e q", e=8),
                            )
                            for p in (0, 1):
                                proj_qk(xk_sb, load_w(pw, wk[p]), kt[p], rc_sb, rs_sb)
                            for p in (0, 1):
                                wv_sb = load_w(pw, wv[p])
                                for qb in range(nkb):
                                    ps = psum_tile()
                                    for e in range(8):
                                        nc.tensor.matmul(
                                            ps[:, 0:FSH],
                                            xv_sb[:, e, qb * 128 : (qb + 1) * 128],
                                            wv_sb[:, e, :],
                                            start=(e == 0),
                                            stop=(e == 7),
                                        )
                                    nc.vector.tensor_copy(
                                        vt[p][:, qb, :, 0:64],
                                        ps[:, 0:FSH].rearrange(
                                            "p (h d) -> p h d", h=HQ
                                        ),
                                    )

                # ---- phase B: attention (needs qt/kt => inside pqk) ----
                # Both paths interleaved piece-by-piece: while one path's exp
                # runs on ScalarE, the PE computes the other path's scores.
                scale = 1.0 / math.sqrt(HEAD_DIM)
                for pair in (0, 1):
                    hA, hB = 2 * pair, 2 * pair + 1
                    for qc in range(nqc):
                        plist = pieces_by_qc[qc]
                        if not plist:
                            continue
                        cacc = {
                            (0, "A"): psum_tile(tag="pC0A"),
                            (0, "B"): psum_tile(tag="pC0B"),
                            (1, "A"): psum_tile(tag="pC1A"),
                            (1, "B"): psum_tile(tag="pC1B"),
                        }
                        nkp = len(plist)
                        for ip, piece in enumerate(plist):
                            kb = piece["kb"]
                            qs = piece["qb0"] * KB
                            n = (piece["qb1"] - piece["qb0"] + 1) * KB
                            rel0 = qs - qc * QC
                            ksl = slice(kb * KB, (kb + 1) * KB)
                            qsl = slice(qs, qs + n)
                            for path in (0, 1):
                                # both heads' scores in one 2-bank psum tile
                                sAB = psum_tile(tag="pS")
                                nc.tensor.matmul(
                                    sAB[:, 0:n],
                                    kt[path][0:64, pair, ksl],
                                    qt[path][0:64, pair, qsl],
                                    start=True,
                                    stop=True,
                                )
                                nc.tensor.matmul(
                                    sAB[:, QC : QC + n],
                                    kt[path][64:128, pair, ksl],
                                    qt[path][64:128, pair, qsl],
                                    start=True,
                                    stop=True,
                                )
                                for qb, ti in piece["mixed"]:
                                    r = qb * KB - qs
                                    for off in (0, QC):
                                        msl = slice(off + r, off + r + KB)
                                        nc.vector.tensor_add(
                                            sAB[:, msl], sAB[:, msl],
                                            mt_sb[:, ti, :],
                                        )
                                eAB = wk_pool.tile(
                                    [128, 2 * QC], BF16, tag="eAB", bufs=4
                                )
                                if n == QC:
                                    nc.scalar.activation(
                                        eAB[:], sAB[:],
                                        mybir.ActivationFunctionType.Exp,
                                        scale=scale,
                                    )
                                else:
                                    nc.scalar.activation(
                                        eAB[:, 0:n], sAB[:, 0:n],
                                        mybir.ActivationFunctionType.Exp,
                                        scale=scale,
                                    )
                                    nc.scalar.activation(
                                        eAB[:, QC : QC + n], sAB[:, QC : QC + n],
                                        mybir.ActivationFunctionType.Exp,
                                        scale=scale,
                                    )
                                for h, off in ((hA, 0), (hB, QC)):
                                    dst = wout[path][
                                        h, piece["poff"] : piece["poff"] + 128 * n
                                    ].rearrange("(p q) -> p q", p=128)
                                    nc.sync.dma_start(
                                        out=dst, in_=eAB[:, off : off + n]
                                    )
                                nc.tensor.matmul(
                                    cacc[(path, "A")][0:65, rel0 : rel0 + n],
                                    vt[path][:, kb, hA, :],
                                    eAB[:, 0:n],
                                    start=(ip == 0),
                                    stop=(ip == nkp - 1),
                                )
                                nc.tensor.matmul(
                                    cacc[(path, "B")][0:65, rel0 : rel0 + n],
                                    vt[path][:, kb, hB, :],
                                    eAB[:, QC : QC + n],
                                    start=(ip == 0),
                                    stop=(ip == nkp - 1),
                                )
                        qsl = slice(qc * QC, (qc + 1) * QC)
                        for half, rows in (("A", slice(0, 64)), ("B", slice(64, 128))):
                            c0 = cacc[(0, half)]
                            c1 = cacc[(1, half)]
                            nc.vector.tensor_copy(ctx[pair][rows, qsl], c0[0:64, :])
                            nc.vector.tensor_mul(
                                ctx[pair][rows, qsl], c1[0:64, :],
                                ctx[pair][rows, qsl],
                            )
                            h = hA if half == "A" else hB
                            nc.vector.tensor_copy(
                                den_ab[0][32 * h : 32 * h + 1, qsl], c0[64:65, :]
                            )
                            nc.vector.tensor_copy(
                                den_ab[1][32 * h : 32 * h + 1, qsl], c1[64:65, :]
                            )

            # ---- phase C: fuse, normalize, output projection ----
            with tc.tile_pool(name="px4", bufs=1) as px4:
                wo_sb = px4.tile([128, 2, N_EMBD], F32R, name="wo_sb")
                nc.sync.dma_start(out=wo_sb[:, 0, :], in_=wo[:, 0:N_EMBD])
                nc.sync.dma_start(out=wo_sb[:, 1, :], in_=wo[:, N_EMBD:])
                # one full-tile reciprocal (free-dim cost only) + row copies
                rr_sb = px4.tile([128, S], F32, name="rr_sb")
                rrcf = px4.tile([128, S], F32, name="rrcf")
                rrc_t = [px4.tile([1, S], F32R, name=f"rrc{h}") for h in range(4)]
                nc.vector.tensor_mul(rr_sb[:], den_ab[0][:], den_ab[1][:])
                with nc.allow_low_precision(reason="unused rows may be junk"):
                    nc.vector.reciprocal(rrcf[:], rr_sb[:])
                for h in range(4):
                    nc.vector.tensor_copy(
                        rrc_t[h][:], rrcf[32 * h : 32 * h + 1, :]
                    )
                for t in (0, 1):
                    for qc in range(nqc):
                        qsl = slice(qc * QC, (qc + 1) * QC)
                        aps = psum_tile()
                        nc.tensor.matmul(
                            aps[:],
                            sel_sb[:, 0:128],
                            rrc_t[2 * t][:, qsl],
                            start=True,
                            stop=False,
                        )
                        nc.tensor.matmul(
                            aps[:],
                            sel_sb[:, 128:256],
                            rrc_t[2 * t + 1][:, qsl],
                            start=False,
                            stop=True,
                        )
                        nc.vector.tensor_mul(ctx[t][:, qsl], ctx[t][:, qsl], aps[:])
                for qb in range(S // 128):
                    for fb2 in (0, 1):
                        ps = psum_tile()
                        for t in (0, 1):
                            nc.tensor.matmul(
                                ps[:],
                                ctx[t][:, qb * 128 : (qb + 1) * 128],
                                wo_sb[:, t, fb2 * QC : (fb2 + 1) * QC],
                                start=(t == 0),
                                stop=(t == 1),
                            )
                        ost = wk_pool.tile([128, QC], F32, tag="ost")
                        nc.scalar.copy(ost[:], ps[:])
                        nc.sync.dma_start(
                            out=outp[
                                qb * 128 : (qb + 1) * 128, fb2 * QC : (fb2 + 1) * QC
                            ],
                            in_=ost[:],
                        )
                for p in (0, 1):
                    for h in range(4):
                        nc.sync.dma_start(
                            out=dent[p * 4 + h : p * 4 + h + 1, :],
                            in_=den_ab[p][32 * h : 32 * h + 1, :],
                        )

    _legalize_waits(nc)
    return nc


# --------------------------------------------------------------------------
# host orchestration
# --------------------------------------------------------------------------
def _ensure_profile_hook():
    """Best-effort: make trace=True work under axon (test-time only)."""
    import sys
    import types

    try:
        from concourse import bass_utils as _bu

        if not getattr(_bu, "_safe_upload_installed", False):
            _orig = _bu.upload_artifacts

            def _safe_upload(tmpdir):
                try:
                    return _orig(tmpdir)
                except Exception:
                    return tmpdir

            _bu.upload_artifacts = _safe_upload
            _bu._safe_upload_installed = True
    except Exception:
        pass
    try:
        from antenv.axon_hooks import get_axon_ntff_profile_hook  # noqa: F401

        return
    except ImportError:
        pass
    try:
        import antenv

        mod = types.ModuleType("antenv.axon_hooks")
        _h = [None]
        mod.get_axon_ntff_profile_hook = lambda: _h[0]
        mod.set_axon_ntff_profile_hook = lambda hook: _h.__setitem__(0, hook)
        sys.modules["antenv.axon_hooks"] = mod
        antenv.axon_hooks = mod
        from trn_agent_boot.trn_boot import _ntff_profile_via_ctypes

        mod.set_axon_ntff_profile_hook(
            _ntff_profile_via_ctypes("/opt/axon/libaxon_pjrt.so")
        )
    except Exception:
        pass


def _interleave_et(arr_t, width):
    """[8*128, width] -> [128, 8*width] with [p, e*width + c] = arr_t[e*128+p, c]"""
    return np.ascontiguousarray(
        arr_t.reshape(8, 128, width).transpose(1, 0, 2).reshape(128, 8 * width)
    )


def _rope_tables(S):
    half = HEAD_DIM // 2
    inv_freq = 1.0 / (10000.0 ** (np.arange(half, dtype=np.float32) / half))
    ang = np.arange(S, dtype=np.float32)[None, :] * inv_freq[:, None]  # [32, S]
    c = np.cos(ang).astype(np.float32)
    s = np.sin(ang).astype(np.float32)
    C = np.tile(c, (4, 1))
    Sm = np.concatenate([-s, s, -s, s], axis=0)
    return C.astype(ml_dtypes.bfloat16), Sm.astype(ml_dtypes.bfloat16)


def kernel(query, key, value, mask, WQ1, WK1, WV1, WQ2, WK2, WV2, WO):
    global LAST_EXEC_NS
    query = np.asarray(query, dtype=np.float32)
    key_ = np.asarray(key, dtype=np.float32)
    value = np.asarray(value, dtype=np.float32)
    mask = np.asarray(mask)
    Ws = [np.asarray(w, dtype=np.float32) for w in (WQ1, WK1, WV1, WQ2, WK2, WV2, WO)]
    WQ1, WK1, WV1, WQ2, WK2, WV2, WO = Ws
    B, S, E = query.shape
    assert E == N_EMBD and B == 2 and S % QC == 0

    mask2d = np.broadcast_to(mask, (1, 1, S, S))[0, 0].astype(bool)
    kstat, mtiles = _classify_mask(mask2d)
    pieces_by_qc, npack = _plan_pieces(kstat, S)
    if _need_zero_tile(pieces_by_qc):
        mtiles = list(mtiles) + [np.full((128, 128), -1e9, np.float32)]
        zidx = len(mtiles) - 1
        for pl in pieces_by_qc:
            for p in pl:
                p["mixed"] = [(qb, ti if ti >= 0 else zidx) for qb, ti in p["mixed"]]
    n_mtiles = len(mtiles)

    key_sig = (S, n_mtiles, tuple(
        (p["kb"], p["qb0"], p["qb1"], tuple(p["mixed"]))
        for pl in pieces_by_qc for p in pl
    ))
    if key_sig not in _prog_cache:
        _prog_cache[key_sig] = _build_program(S, pieces_by_qc, npack, n_mtiles)
    nc = _prog_cache[key_sig]

    nkb = S // KB
    ropeC, ropeS = _rope_tables(S)
    sel = np.zeros((1, 256), np.float32)
    sel[0, 0:64] = 1.0
    sel[0, 192:256] = 1.0
    onesv = np.ones((128, nkb * HQ), ml_dtypes.bfloat16)
    mt_flat = (
        np.ascontiguousarray(np.stack(mtiles, axis=1).reshape(128, n_mtiles * 128))
        if n_mtiles
        else None
    )

    in_maps = []
    for core in range(N_CORES):
        b, hq = core // 4, core % 4
        sh = slice(hq * FSH, (hq + 1) * FSH)
        im = {
            "xq": _interleave_et(query[b].T, S).astype(ml_dtypes.bfloat16),
            "xk": _interleave_et(key_[b].T, S).astype(ml_dtypes.bfloat16),
            "xv": _interleave_et(value[b].T, S).astype(ml_dtypes.bfloat16),
            "wq1": _interleave_et(WQ1[sh].T, FSH).astype(ml_dtypes.bfloat16),
            "wq2": _interleave_et(WQ2[sh].T, FSH).astype(ml_dtypes.bfloat16),
            "wk1": _interleave_et(WK1[sh].T, FSH).astype(ml_dtypes.bfloat16),
            "wk2": _interleave_et(WK2[sh].T, FSH).astype(ml_dtypes.bfloat16),
            "wv1": _interleave_et(WV1[sh].T, FSH).astype(ml_dtypes.bfloat16),
            "wv2": _interleave_et(WV2[sh].T, FSH).astype(ml_dtypes.bfloat16),
            "wo": np.ascontiguousarray(
                WO[:, sh].T.reshape(2, 128, N_EMBD).transpose(1, 0, 2).reshape(128, -1)
            ),
            "ropec": ropeC,
            "ropes": ropeS,
            "sel": sel,
            "onesv": onesv,
        }
        if mt_flat is not None:
            im["mtiles"] = mt_flat
        in_maps.append(im)

    if PROFILE:
        _ensure_profile_hook()
    try:
        res = run_bass_kernel_spmd(nc, in_maps, list(range(N_CORES)), trace=PROFILE)
    except Exception:
        if not PROFILE:
            raise
        res = run_bass_kernel_spmd(nc, in_maps, list(range(N_CORES)), trace=False)
    if res.exec_time_ns is not None:
        LAST_EXEC_NS = res.exec_time_ns

    # ---- assemble ----
    out = np.zeros((B, S, N_EMBD), np.float32)
    w1 = np.zeros((B, N_HEAD, S, S), np.float32)
    w2 = np.zeros((B, N_HEAD, S, S), np.float32)
    all_pieces = [p for pl in pieces_by_qc for p in pl]
    for core in range(N_CORES):
        r = res.results[core]
        b, hq = core // 4, core % 4
        den = r["den"]  # [8, S]
        for path, (wp_name, wfull) in enumerate((("w1p", w1), ("w2p", w2))):
            wp = r[wp_name]
            for h in range(HQ):
                H = hq * HQ + h
                recip = 1.0 / den[path * 4 + h]  # [S]
                for p in all_pieces:
                    n = (p["qb1"] - p["qb0"] + 1) * KB
                    qs = p["qb0"] * KB
                    kb = p["kb"]
                    blockT = np.asarray(
                        wp[h, p["poff"] : p["poff"] + 128 * n],
                        dtype=np.float32,
                    ).reshape(128, n)
                    wfull[b, H, qs : qs + n, kb * KB : (kb + 1) * KB] = (
                        blockT.T * recip[qs : qs + n, None]
                    )
        out[b] += r["outp"]
    return out, w1, w2
